# revision 8
# baseline (speedup 1.0000x reference)
"""Trainium2 Bass kernel for nn_AttnDecoder (GRU + Bahdanau attention decoder).

Strategy: batch-parallel over B=64 -> 8 rows/core, no collectives.
The sequential 30-step recurrence (tiny: ~5% of FLOPs) runs on host;
the dominant vocab projection [1920, 2048] @ [2048, 32000] + log_softmax
(252 GFLOP, 262 MB weights) runs on the 8 NeuronCores in bf16 with f32
PSUM accumulation and a fused online sum-exp.
"""

import sys

sys.path.insert(0, "/opt/trn_rl_repo")
sys.path.insert(0, "/opt/pypackages")

import ml_dtypes
import numpy as np

MAX_LENGTH = 30
SOS_TOKEN = 2
V, E, H = 32000, 512, 512
B, S = 64, 128
NCORES = 8
BC = B // NCORES          # batch rows per core
ROWS = BC * MAX_LENGTH    # fc rows per core = 240
F = E + 3 * H             # fc feature dim = 2048
KT = F // 128             # k tiles = 16
NT = 512                  # vocab tile size
BF16 = ml_dtypes.bfloat16


def _host_recurrence(encoder_outputs, encoder_hidden, target_tensor, embedding,
                     wa, ua, va, w_ih, w_hh, b_ih, b_hh):
    """Sequential GRU+attention recurrence in f32 numpy. Returns feats [B, T, F]."""
    b = encoder_outputs.shape[0]
    toks = np.concatenate(
        [np.full((b, 1), SOS_TOKEN, target_tensor.dtype), target_tensor[:, :-1]],
        axis=1).T  # [T, B]
    enc_ua = np.einsum('bsk,hk->bsh', encoder_outputs, ua)  # [B, S, H]
    h = encoder_hidden.astype(np.float32)
    feats = np.empty((b, MAX_LENGTH, F), np.float32)
    waT = wa.T.copy()
    w_ihT = w_ih.T.copy()
    w_hhT = w_hh.T.copy()
    for t in range(MAX_LENGTH):
        emb = embedding[toks[t]]                                   # [B, E]
        energy = np.tanh((h @ waT)[:, None, :] + enc_ua)           # [B, S, H]
        scores = energy @ va[0]                                    # [B, S]
        sm = np.exp(scores - scores.max(axis=-1, keepdims=True))
        attw = sm / sm.sum(axis=-1, keepdims=True)
        context = np.einsum('bs,bsd->bd', attw, encoder_outputs)   # [B, 2H]
        x = np.concatenate([emb, context], axis=-1)
        gi = x @ w_ihT + b_ih
        gh = h @ w_hhT + b_hh
        i_r, i_z, i_n = np.split(gi, 3, axis=-1)
        h_r, h_z, h_n = np.split(gh, 3, axis=-1)
        r = 1.0 / (1.0 + np.exp(-(i_r + h_r)))
        z = 1.0 / (1.0 + np.exp(-(i_z + h_z)))
        n = np.tanh(i_n + r * h_n)
        h = (1.0 - z) * n + z * h
        feats[:, t, :E] = emb
        feats[:, t, E:E + H] = h
        feats[:, t, E + H:] = context
    return feats


_CACHED = {}


def _build_nc():
    """Build the Bass program: fc matmul + log_softmax for one core's shard."""
    import concourse.bacc as bacc
    import concourse.tile as tile
    import concourse.mybir as mybir

    nc = bacc.Bacc(None, target_bir_lowering=False)
    dt = mybir.dt
    AF = mybir.ActivationFunctionType
    Alu = mybir.AluOpType

    featsT = nc.dram_tensor("featsT", [128, KT, ROWS], dt.bfloat16,
                            kind="ExternalInput")       # [p, ko, m] = feats.T
    wT = nc.dram_tensor("wT", [KT, 128, V], dt.bfloat16,
                        kind="ExternalInput")           # [(ko p), n] = fc_w.T
    fcb = nc.dram_tensor("fcb", [1, V], dt.bfloat16, kind="ExternalInput")
    out = nc.dram_tensor("out", [ROWS, V], dt.float32, kind="ExternalOutput")

    m_tiles = [(0, 128), (128, ROWS - 128)]             # (start, size)
    n_tiles = []
    n0 = 0
    while n0 < V:
        n_tiles.append((n0, min(NT, V - n0)))
        n0 += NT

    with tile.TileContext(nc) as tc:
        with (
            tc.tile_pool(name="weights", bufs=2) as wpool,
            tc.tile_pool(name="feats", bufs=1) as fpool,
            tc.tile_pool(name="logits", bufs=2) as lpool,
            tc.tile_pool(name="psum", bufs=8, space="PSUM") as ppool,
            tc.tile_pool(name="small", bufs=4) as spool,
            tc.tile_pool(name="scratch", bufs=3) as scpool,
            tc.tile_pool(name="stage", bufs=4) as stpool,
        ):
            ft = fpool.tile([128, KT, ROWS], dt.bfloat16, tag="ft")
            nc.sync.dma_start(out=ft[:], in_=featsT[:])
            ones = fpool.tile([1, 128], dt.bfloat16, tag="ones")
            nc.vector.memset(ones[:], 1.0)

            logits = []
            sums = []
            for mi, (m0, ms) in enumerate(m_tiles):
                lg = lpool.tile([128, V], dt.bfloat16, tag="logits")
                logits.append(lg)
                rs = spool.tile([128, 1], dt.float32, tag=f"rs{mi}")
                nc.vector.memset(rs[:ms], 0.0)
                sums.append(rs)

            # pass 1: matmul tiles + bias, store bf16 logits, accumulate sum(exp)
            for ni, (nst, nsz) in enumerate(n_tiles):
                wt = wpool.tile([128, KT, NT], dt.bfloat16, tag="w")
                nc.sync.dma_start(
                    out=wt[:, :, :nsz],
                    in_=wT[:, :, nst:nst + nsz].rearrange("ko p n -> p ko n"),
                )
                bt = spool.tile([1, NT], dt.bfloat16, tag="bt")
                nc.sync.dma_start(out=bt[:1, :nsz], in_=fcb[:1, nst:nst + nsz])
                for mi, (m0, ms) in enumerate(m_tiles):
                    ps = ppool.tile([128, NT], dt.float32, tag="ps")
                    # bias row: ones[1,ms].T @ bias[1,nsz] outer product
                    nc.tensor.matmul(
                        out=ps[:ms, :nsz], lhsT=ones[:1, :ms], rhs=bt[:1, :nsz],
                        start=True, stop=False,
                    )
                    for k in range(KT):
                        nc.tensor.matmul(
                            out=ps[:ms, :nsz],
                            lhsT=ft[:, k, m0:m0 + ms],
                            rhs=wt[:, k, :nsz],
                            start=False,
                            stop=(k == KT - 1),
                        )
                    nc.vector.tensor_copy(
                        out=logits[mi][:ms, nst:nst + nsz], in_=ps[:ms, :nsz])
                    # fused exp + row-sum accumulation (logits are ~|x|<1,
                    # so exp without max subtraction is numerically safe)
                    ex = scpool.tile([128, NT], dt.bfloat16, tag="ex")
                    cs = spool.tile([128, 1], dt.float32, tag="cs")
                    nc.scalar.activation(
                        out=ex[:ms, :nsz], in_=ps[:ms, :nsz], func=AF.Exp,
                        accum_out=cs[:ms],
                    )
                    nc.vector.tensor_add(
                        out=sums[mi][:ms], in0=sums[mi][:ms], in1=cs[:ms])

            # pass 2: shift = ln(sum); out = logits - shift
            for mi, (m0, ms) in enumerate(m_tiles):
                sh = spool.tile([128, 1], dt.float32, tag=f"sh{mi}")
                nc.scalar.activation(
                    out=sh[:ms], in_=sums[mi][:ms], func=AF.Ln)
                for ni, (nst, nsz) in enumerate(n_tiles):
                    st = stpool.tile([128, NT], dt.float32, tag="st")
                    nc.vector.tensor_scalar(
                        out=st[:ms, :nsz],
                        in0=logits[mi][:ms, nst:nst + nsz],
                        scalar1=sh[:ms],
                        scalar2=None,
                        op0=Alu.subtract,
                    )
                    nc.sync.dma_start(
                        out=out[m0:m0 + ms, nst:nst + nsz], in_=st[:ms, :nsz])
    nc.compile()
    return nc


def kernel(encoder_outputs, encoder_hidden, target_tensor, embedding, wa, ua, va,
           w_ih, w_hh, b_ih, b_hh, fc_w, fc_b):
    from concourse.bass_utils import run_bass_kernel_spmd

    encoder_outputs = np.asarray(encoder_outputs, np.float32)
    encoder_hidden = np.asarray(encoder_hidden, np.float32)
    target_tensor = np.asarray(target_tensor)
    feats = _host_recurrence(
        encoder_outputs, encoder_hidden, target_tensor,
        np.asarray(embedding, np.float32), np.asarray(wa, np.float32),
        np.asarray(ua, np.float32), np.asarray(va, np.float32),
        np.asarray(w_ih, np.float32), np.asarray(w_hh, np.float32),
        np.asarray(b_ih, np.float32), np.asarray(b_hh, np.float32))

    # weights layout [(ko p), n] -> [ko, 128, V], shared by all cores
    wT = np.ascontiguousarray(np.asarray(fc_w, np.float32).T).astype(BF16)
    wT = wT.reshape(KT, 128, V)
    fcb = np.asarray(fc_b, np.float32).astype(BF16).reshape(1, V)

    in_maps = []
    for c in range(NCORES):
        fc_feats = feats[c * BC:(c + 1) * BC].reshape(ROWS, F)   # rows = b*T + t
        ftT = np.ascontiguousarray(fc_feats.T).astype(BF16)      # [F, ROWS]
        ftT = np.ascontiguousarray(
            ftT.reshape(KT, 128, ROWS).transpose(1, 0, 2))       # [p, ko, m]
        in_maps.append({"featsT": ftT, "wT": wT, "fcb": fcb})

    if "nc" not in _CACHED:
        _CACHED["nc"] = _build_nc()
    import time as _time
    t0 = _time.time()
    res = run_bass_kernel_spmd(_CACHED["nc"], in_maps, core_ids=list(range(NCORES)))
    _CACHED["spmd_s"] = _time.time() - t0
    _CACHED["last_result"] = res

    out = np.empty((B, MAX_LENGTH, V), np.float32)
    for c in range(NCORES):
        out[c * BC:(c + 1) * BC] = res.results[c]["out"].reshape(BC, MAX_LENGTH, V)
    return out
